# revision 50
# baseline (speedup 1.0000x reference)
"""Bi-directional minGRU kernel for Trainium2 (8 NeuronCores, Bass/Tile).

Strategy
--------
Data-parallel over batch: B=256 examples sharded 32 per core. Per example all
tensors live in feature-major layout [feature->partition, time->free]; fp16
on-chip (PE fp16 matmul = 1 cycle/row vs 4 for fp32; DVE 2x/4x fp16 modes).

Rows 0-63 carry the forward direction in normal time order, rows 64-127 the
backward direction in reversed time order (reversal is free: backward gate
matmuls read a negative-stride view of the input), so the whole minGRU
recurrence is ONE hardware tensor_tensor_scan [128, 2048] on DVE.

Host-side (numpy, fp64 then cast) the linear chains are fused:
    gz = (wz@proj[:, :3]) @ x3 + (wz@proj[:, 3:]@te_w2) @ r + bias
and the masked-position fixup  h_apply = m*pre + (1-m)*final  commutes with
the head matmul:  W@h_apply = W@(m*(pre-final)) + (W@final folded into bias).
The host also ships a pre-reversed mask row so the on-chip mask broadcast is
a forward-stride DMA (a reversed broadcast DMA degenerates to per-element
descriptors, ~40x cost — this was the dominant cost of the fp32 baseline).

The time encoder r = relu(w1*t + b1) is an outer product: computed on DVE
with per-partition scalars from a broadcast copy of the t row (no matmul,
no psum). The program is software-pipelined: per iteration it emits the DMA
stage for example e+1, pre-scan compute for e, and post-scan compute for
e-1, so every engine's in-order queue matches data readiness.
"""
import os
import sys

for _p in ("/opt/trn_rl_repo", "/root/.axon_site/_ro/trn_rl_repo"):
    if os.path.isdir(_p) and _p not in sys.path:
        sys.path.insert(0, _p)

import numpy as np
from contextlib import ExitStack

import concourse.bacc as bacc
import concourse.tile as tile
import concourse.mybir as mybir
from concourse.bass_utils import run_bass_kernel_spmd

F32 = mybir.dt.float32
F16 = mybir.dt.float16
AF = mybir.ActivationFunctionType
OP = mybir.AluOpType

B, L, H, TE = 256, 2048, 64, 64
NCORES = 8
BS = B // NCORES          # examples per core

# fp16 weight tile column layout [128, NWH]
_C_ZF = 0                 # gate lhsTs [67, 64] each
_C_ZB = 64
_C_HF = 128
_C_HB = 192
_C_W1FB = 256             # rows 0-63: W1f^T, rows 64-127: W1b^T  [*, 128]
_C_W1T2 = 384             # (W1t@te_w2)^T rows 0-63  [64, 128]
_C_W1B0 = 512             # W1b^T at rows 64-127, rows 0-63 zero [128, 128]:
                          # lets the Erev matmul run as K=128 at tile
                          # position (0,0) — mixing (0,0) and (64,0) matmuls
                          # in one psum accumulation group crashes the HW
_C_W2B = 640              # [128, 193] zero band, gh_w2 at col 96; the lhsT
                          # view [96-32g : 96-32g+97] puts w2 at local col
                          # 32g so each head2 matmul writes M=97 rows
                          # (zeros except row 32g) keeping psum initialized
NWH = 833
# fp32 scalar-column tile [128, NWF]
_F_ZB = 0                 # gate z bias (fwd rows 0-63, bwd 64-127)
_F_HB = 1                 # gate h bias
_F_HEADB = 2              # head bias (gh_b1 + W1t@te_b2)
_F_W1 = 3                 # te_w1[:,0] rows 0-63
_F_B1 = 4                 # te_b1 rows 0-63
NWF = 5

_cache = {}


def _pack_weights(inp):
    g = {k: np.asarray(v, np.float64) for k, v in inp.items()}
    wh = np.zeros((128, NWH), np.float64)
    wf = np.zeros((128, NWF), np.float64)

    def fuse(proj_w, proj_b, wz, bz, wh_, bh):
        P3 = proj_w[:, :3]
        Pte_te2 = proj_w[:, 3:] @ g["te_w2"]
        pbias = proj_w[:, 3:] @ g["te_b2"] + proj_b
        # device feature rows are ordered [r(64), mask, x1, x2]
        reord = np.stack([P3[:, 2], P3[:, 0], P3[:, 1]], axis=1)
        return (
            np.concatenate([wz @ Pte_te2, wz @ reord], axis=1),   # (64, 67)
            wz @ pbias + bz,
            np.concatenate([wh_ @ Pte_te2, wh_ @ reord], axis=1),
            wh_ @ pbias + bh,
        )

    Zf, zbf, Hf, hbf = fuse(g["fproj_w"], g["fproj_b"], g["fwz"], g["fbz"],
                            g["fwh"], g["fbh"])
    Zb, zbb, Hb, hbb = fuse(g["bproj_w"], g["bproj_b"], g["bwz"], g["bbz"],
                            g["bwh"], g["bbh"])
    wh[0:67, _C_ZF:_C_ZF + 64] = Zf.T
    wh[0:67, _C_ZB:_C_ZB + 64] = Zb.T
    wh[0:67, _C_HF:_C_HF + 64] = Hf.T
    wh[0:67, _C_HB:_C_HB + 64] = Hb.T
    wf[0:64, _F_ZB] = zbf
    wf[64:128, _F_ZB] = zbb
    wf[0:64, _F_HB] = hbf
    wf[64:128, _F_HB] = hbb
    # head
    W1f = g["gh_w1"][:, :64]
    W1b = g["gh_w1"][:, 64:128]
    W1t = g["gh_w1"][:, 128:192]
    wh[0:64, _C_W1FB:_C_W1FB + 128] = W1f.T
    wh[64:128, _C_W1FB:_C_W1FB + 128] = W1b.T
    wh[0:64, _C_W1T2:_C_W1T2 + 128] = (W1t @ g["te_w2"]).T
    wh[64:128, _C_W1B0:_C_W1B0 + 128] = W1b.T
    wh[0:128, _C_W2B + 96] = g["gh_w2"][0]
    wf[0:128, _F_HEADB] = g["gh_b1"] + W1t @ g["te_b2"]
    # te first layer as per-partition scalars
    wf[0:64, _F_W1] = g["te_w1"][:, 0]
    wf[0:64, _F_B1] = g["te_b1"]
    return (np.ascontiguousarray(wh, np.float16),
            np.ascontiguousarray(wf, np.float32),
            np.float32(g["gh_b2"][0]))


def _build_program(bs=BS, reps=1):
    nc = bacc.Bacc("TRN2", num_devices=NCORES, debug=False)
    wh_d = nc.dram_tensor("wh", [128, NWH], F16, kind="ExternalInput")
    wf_d = nc.dram_tensor("wf", [128, NWF], F32, kind="ExternalInput")
    # input rows per example: [r(64 rows, host-precomputed time encoding),
    # mask, x1s, x2s, mask_reversed]
    inx_d = nc.dram_tensor("inx", [bs, 68, L], F16, kind="ExternalInput")
    out_d = nc.dram_tensor("out", [bs, L], F32, kind="ExternalOutput")

    with tile.TileContext(nc) as tc, ExitStack() as ctx:
        wpool = ctx.enter_context(tc.tile_pool(name="w", bufs=1))
        pool = ctx.enter_context(tc.tile_pool(name="p", bufs=3))
        pool3 = ctx.enter_context(tc.tile_pool(name="p3", bufs=3))
        pool4 = ctx.enter_context(tc.tile_pool(name="p4", bufs=5))
        spool = ctx.enter_context(tc.tile_pool(name="s", bufs=2))
        ps_pre = ctx.enter_context(tc.tile_pool(name="pre", bufs=2,
                                                space="PSUM"))
        ps_post = ctx.enter_context(tc.tile_pool(name="post", bufs=2,
                                                 space="PSUM"))
        ps_p2 = ctx.enter_context(tc.tile_pool(name="p2", bufs=2,
                                               space="PSUM"))

        _hid_group = {}
        wth = wpool.tile([128, NWH], F16, tag="wth", name="wth")
        wtf = wpool.tile([128, NWF], F32, tag="wtf", name="wtf")
        nc.sync.dma_start(wth[:], wh_d.ap()[:])
        nc.sync.dma_start(wtf[:], wf_d.ap()[:])
        inx = inx_d.ap()
        w1c = wtf[0:64, _F_W1:_F_W1 + 1]
        b1c = wtf[0:64, _F_B1:_F_B1 + 1]
        zbc = wtf[:, _F_ZB:_F_ZB + 1]
        hbc = wtf[:, _F_HB:_F_HB + 1]
        hdc = wtf[:, _F_HEADB:_F_HEADB + 1]

        st = {}  # per-example live tiles

        def emit_dma(e):
            s = st.setdefault(e, {})
            xr = pool3.tile([128, L], F16, tag="xr", name=f"xr{e}")
            nc.sync.dma_start(xr[0:67, :], inx[e, 0:67, :])
            m128 = pool3.tile([128, L], F16, tag="m128", name=f"m128_{e}")
            nc.sync.dma_start(m128[0:64, :],
                              inx[e, 64:65, :].broadcast_to((64, L)))
            nc.sync.dma_start(m128[64:128, :],
                              inx[e, 67:68, :].broadcast_to((64, L)))
            s["xr"], s["m128"] = xr, m128

        def emit_pre(e):
            s = st[e]
            xr = s["xr"]
            xrev = xr[0:67, ::-1]
            # gates: fwd rows 0-63 (normal order), bwd rows 64-127 (time
            # reversed via the negative-stride rhs view)
            Z = pool.tile([128, L], F16, tag="Z", name=f"Z{e}")
            TH = pool.tile([128, L], F16, tag="TH", name=f"TH{e}")
            A = pool.tile([128, L], F16, tag="A", name=f"A{e}")
            Bt = pool.tile([128, L], F16, tag="Bt", name=f"Bt{e}")
            for h in range(2):
                hs = slice(h * 1024, (h + 1) * 1024)
                psZ = ps_pre.tile([128, 1024], F32, tag="pre",
                                  name=f"psZ{e}_{h}")
                # matmuls grouped by stationary weights (fewer PE reloads)
                for c in range(2):
                    cs = slice(h * 1024 + c * 512, h * 1024 + (c + 1) * 512)
                    nc.tensor.matmul(psZ[0:64, c * 512:(c + 1) * 512],
                                     wth[0:67, _C_ZF:_C_ZF + 64],
                                     xr[0:67, cs], start=True, stop=True,
                                     tile_position=(0, 0))
                for c in range(2):
                    cs = slice(h * 1024 + c * 512, h * 1024 + (c + 1) * 512)
                    nc.tensor.matmul(psZ[64:128, c * 512:(c + 1) * 512],
                                     wth[0:67, _C_ZB:_C_ZB + 64],
                                     xrev[:, cs], start=True, stop=True,
                                     tile_position=(0, 64))
                nc.scalar.activation(Z[:, hs], psZ[:], AF.Sigmoid, bias=zbc)
                psT = ps_pre.tile([128, 1024], F32, tag="pre",
                                  name=f"psT{e}_{h}")
                for c in range(2):
                    cs = slice(h * 1024 + c * 512, h * 1024 + (c + 1) * 512)
                    nc.tensor.matmul(psT[0:64, c * 512:(c + 1) * 512],
                                     wth[0:67, _C_HF:_C_HF + 64],
                                     xr[0:67, cs], start=True, stop=True,
                                     tile_position=(0, 0))
                for c in range(2):
                    cs = slice(h * 1024 + c * 512, h * 1024 + (c + 1) * 512)
                    nc.tensor.matmul(psT[64:128, c * 512:(c + 1) * 512],
                                     wth[0:67, _C_HB:_C_HB + 64],
                                     xrev[:, cs], start=True, stop=True,
                                     tile_position=(0, 64))
                nc.scalar.activation(TH[:, hs], psT[:], AF.Tanh, bias=hbc)
                # scan inputs a = 1-z, b = z*th, emitted per half so they
                # finish right behind the last tanh; b's tail on GPSIMD
                nc.vector.tensor_scalar(A[:, hs], Z[:, hs], -1.0, 1.0,
                                        OP.mult, OP.add)
                dsz = h * 1024 + 768
                nc.vector.tensor_tensor(Bt[:, h * 1024:dsz],
                                        Z[:, h * 1024:dsz],
                                        TH[:, h * 1024:dsz], OP.mult)
                nc.gpsimd.tensor_tensor(Bt[:, dsz:(h + 1) * 1024],
                                        Z[:, dsz:(h + 1) * 1024],
                                        TH[:, dsz:(h + 1) * 1024], OP.mult)
            s["A"], s["Bt"] = A, Bt

        def emit_scan(e):
            s = st[e]
            A, Bt = s["A"], s["Bt"]
            # ONE scan for both directions; Hs[:, 0:L] = pre-states
            Hs = pool.tile([128, L + 1], F16, tag="Hs", name=f"Hs{e}")
            nc.vector.memset(Hs[:, 0:1], 0.0)
            nc.vector.tensor_tensor_scan(Hs[:, 1:L + 1], A[:], Bt[:],
                                         0.0, OP.mult, OP.add)
            s["Hs"] = Hs

        def emit_post_early(e):
            # the cheap DVE prep for example e's head stage — emitted BEFORE
            # the next example's scan so PE's head matmuls aren't stuck
            # behind 2+ us of unrelated DVE work
            s = st[e]
            m128, Hs = s["m128"], s["Hs"]
            fin32 = spool.tile([128, 1], F32, tag="fin32", name=f"fin32_{e}")
            nc.vector.tensor_copy(fin32[:], Hs[:, L - 1:L])
            psB = ps_post.tile([128, 512], F32, tag="post", name=f"psB{e}")
            nc.tensor.matmul(psB[:, 0:1], wth[:, _C_W1FB:_C_W1FB + 128],
                             Hs[:, L - 1:L], start=True, stop=True)
            sbb = spool.tile([128, 1], F32, tag="sbb", name=f"sbb{e}")
            nc.scalar.activation(sbb[:], psB[:, 0:1], AF.Identity, bias=hdc)
            # masked fixup E = m * (pre - fin); the multiply is split by
            # column halves across DVE and GPSIMD
            Dt = pool.tile([128, L], F16, tag="Dt", name=f"Dt{e}")
            nc.vector.tensor_scalar(Dt[:], Hs[:, 0:L], fin32[:], None,
                                    OP.subtract)
            Et = pool.tile([128, L], F16, tag="Et", name=f"Et{e}")
            nc.vector.tensor_tensor(Et[:], Dt[:], m128[:], OP.mult)
            s["Et"], s["sbb"] = Et, sbb

        def emit_post(e):
            s = st[e]
            xr, Et, sbb = s["xr"], s["Et"], s["sbb"]
            Erev = Et[0:128, ::-1]
            # head layer 1 (all matmuls at tile position (0,0); the Erev one
            # runs as K=128 with zeroed top rows — see _C_W1B0). Two 512
            # chunks in flight, matmuls grouped by stationary weights.
            hid = pool4.tile([128, L], F16, tag="hid", name=f"hid{e}")
            for p in range(2):
                psHs = [ps_post.tile([128, 512], F32, tag="post",
                                     name=f"psH{e}_{2 * p + i}")
                        for i in range(2)]
                for i in range(2):
                    cs = slice((2 * p + i) * 512, (2 * p + i + 1) * 512)
                    nc.tensor.matmul(psHs[i][:],
                                     wth[0:64, _C_W1FB:_C_W1FB + 128],
                                     Et[0:64, cs], start=True, stop=False,
                                     tile_position=(0, 0))
                for i in range(2):
                    cs = slice((2 * p + i) * 512, (2 * p + i + 1) * 512)
                    nc.tensor.matmul(psHs[i][:],
                                     wth[:, _C_W1B0:_C_W1B0 + 128],
                                     Erev[:, cs], start=False, stop=False,
                                     tile_position=(0, 0))
                for i in range(2):
                    cs = slice((2 * p + i) * 512, (2 * p + i + 1) * 512)
                    nc.tensor.matmul(psHs[i][:],
                                     wth[0:64, _C_W1T2:_C_W1T2 + 128],
                                     xr[0:64, cs], start=False, stop=True,
                                     tile_position=(0, 0))
                    nc.scalar.activation(hid[:, cs], psHs[i][:], AF.Relu,
                                         bias=sbb[:])
            # head layer 2: deferred per 4-example group; each 512 chunk's
            # psum tile accumulates the 4 examples at rows 0/32/64/96
            eg = e % 4
            _hid_group[eg] = hid
            if eg == 3:
                for p in range(2):
                    psPs = [ps_p2.tile([128, 512], F32, tag="p2",
                                       name=f"psP{e}_{2 * p + i}")
                            for i in range(2)]
                    for g in range(4):
                        w2v = wth[:, _C_W2B + 96 - 32 * g:
                                  _C_W2B + 96 - 32 * g + 97]
                        for i in range(2):
                            cs = slice((2 * p + i) * 512,
                                       (2 * p + i + 1) * 512)
                            nc.tensor.matmul(psPs[i][0:97, :], w2v,
                                             _hid_group[g][:, cs],
                                             start=(g == 0), stop=(g == 3))
                    for i in range(2):
                        cs = slice((2 * p + i) * 512, (2 * p + i + 1) * 512)
                        pg = spool.tile([128, 512], F32, tag="pg",
                                        name=f"pg{e}_{2 * p + i}")
                        if i == 0:
                            nc.scalar.activation(pg[0:97, :], psPs[i][0:97, :],
                                                 AF.Copy)
                        else:
                            nc.vector.tensor_copy(pg[0:97, :], psPs[i][0:97, :])
                        nc.sync.dma_start(out_d.ap()[e - 3:e + 1, cs],
                                          pg[0:128:32, :])
            st.pop(e - 4, None)

        def emit_all():
            emit_dma(0)
            for it in range(bs + 1):
                if it + 1 < bs:
                    emit_dma(it + 1)
                if it < bs:
                    emit_pre(it)
                if it >= 1:
                    emit_post_early(it - 1)
                if it < bs:
                    emit_scan(it)
                if it >= 1:
                    emit_post(it - 1)

        for _rep in range(reps):
            emit_all()
            st.clear()

    nc.compile()
    return nc


def _pack_host_inputs(x, t, mask_token, te_w1, te_b1):
    x = np.asarray(x, np.float32)
    t = np.asarray(t, np.float32)
    tok = np.asarray(mask_token, np.float32)
    xT = np.swapaxes(x, 1, 2)                    # (B, 3, L)
    mask = xT[:, 2:3, :]
    x12 = np.where(mask == 0, tok.reshape(1, 2, 1), xT[:, 0:2, :])
    mrev = mask[:, :, ::-1]
    # host-precomputed time encoding r = relu(w1*t + b1): (B, 64, L)
    w1 = np.asarray(te_w1, np.float32)[:, 0]
    b1 = np.asarray(te_b1, np.float32)
    r = np.maximum(w1[None, :, None] * t[:, None, :, 0]
                   + b1[None, :, None], 0.0)
    return np.ascontiguousarray(
        np.concatenate([r, mask, x12, mrev], axis=1).astype(np.float16))


def kernel(x, t, mask_token,
           te_w1, te_b1, te_w2, te_b2,
           fproj_w, fproj_b, bproj_w, bproj_b,
           fwz, fbz, fwh, fbh,
           bwz, bbz, bwh, bbh,
           gh_w1, gh_b1, gh_w2, gh_b2):
    inp = dict(te_w1=te_w1, te_b1=te_b1, te_w2=te_w2, te_b2=te_b2,
               fproj_w=fproj_w, fproj_b=fproj_b, bproj_w=bproj_w,
               bproj_b=bproj_b, fwz=fwz, fbz=fbz, fwh=fwh, fbh=fbh,
               bwz=bwz, bbz=bbz, bwh=bwh, bbh=bbh,
               gh_w1=gh_w1, gh_b1=gh_b1, gh_w2=gh_w2, gh_b2=gh_b2)
    wh, wf, b2 = _pack_weights(inp)
    inx = _pack_host_inputs(x, t, mask_token, te_w1, te_b1)

    if "nc" not in _cache:
        _cache["nc"] = _build_program()
    nc = _cache["nc"]

    in_maps = [
        {"wh": wh, "wf": wf, "inx": inx[c * BS:(c + 1) * BS]}
        for c in range(NCORES)
    ]
    res = run_bass_kernel_spmd(nc, in_maps, core_ids=list(range(NCORES)))
    out = np.concatenate([res.results[c]["out"] for c in range(NCORES)],
                         axis=0)
    return (out + b2).reshape(B, L, 1).astype(np.float32)
